# revision 23
# baseline (speedup 1.0000x reference)
"""Trainium2 Bass kernel for nn_HGNNExpertCoupler (B=8, L=1024, E=8, D=512).

Math: the all-pairs hypergraph operator D^-1 H B^-1 H^T has unit column
sums, so it preserves the expert-mean, and the whole network collapses to

    out = LN(gelu(mean_E(x) @ Wz^T + bz)) * gamma + beta
    Wz  = Wc @ W1 @ W0,  bz = (b0 @ W1^T + b1) @ Wc^T + bc

Per-core layout (data parallel on B, one batch row per core, 1024 tokens):

  x is staged on host as fp16, d-major, one DRAM plane per token phase
  (uneven 384/256/256/128 split so the tail chain after the last byte is
  short).  Loads are plain full-rate HWDGE transfers on the sync queue
  (2-6 KB descriptors); wzt rides the otherwise-idle gpsimd queue.  A
  burst of dummy matmuls on wzt pre-warms the PE HAM clock gate while
  the stream is in flight.  The expert reduction is a 2-level fp16
  tensor_tensor tree on DVE (8 -> 4 -> 2 partial sums); the final 2-way
  fold is absorbed into the matmul accumulation (8 MMs per 128-token
  group).  ACT applies Gelu (accum_out = per-token sum z) and Square
  (accum_out = sum z^2, output recycled into the group's psum bank);
  both live in the gelu_and_others table set (one table load, warmed at
  t~0).  LayerNorm finishes on DVE: per-phase batched quake-rsqrt + 1
  Newton step, then one fp16 tensor_scalar normalize per group.
  Outputs are written fp16 and upcast on host.
"""

import os
import sys

import numpy as np

for _p in ("/opt/trn_rl_repo", "/opt/trn_rl_repo/pypackages",
           "/root/.axon_site/_ro/trn_rl_repo",
           "/root/.axon_site/_ro/pypackages"):
    if os.path.isdir(_p) and _p not in sys.path:
        sys.path.append(_p)

from contextlib import ExitStack

import concourse.bass as bass
import concourse.tile as tile
from concourse import bacc, mybir
from concourse.bass_utils import run_bass_kernel_spmd

FP = mybir.dt.float32
F16 = mybir.dt.float16
I32 = mybir.dt.int32

B, L, E, D = 8, 1024, 8, 512
KT = D // 128                     # 4 contraction k-blocks
NPH = [384, 256, 256, 128]        # tokens per phase
OFF = [0, 384, 640, 896]
PH = len(NPH)
LN_EPS = 1e-5
N_CORES = 8

_CACHE = {}


def _build(use_gb: bool, use_bz: bool):
    nc = bacc.Bacc("TRN2", target_bir_lowering=False, debug=False,
                   num_devices=N_CORES)

    x_d = [nc.dram_tensor(f"x{h}", [D, E * NPH[h]], F16,
                          kind="ExternalInput").ap()
           for h in range(PH)]
    wzt_d = nc.dram_tensor("wzt", [KT, 128, D], F16, kind="ExternalInput").ap()
    if use_gb:
        gb_d = nc.dram_tensor("gb", [128, 2 * D], FP, kind="ExternalInput").ap()
    if use_bz:
        bz_d = nc.dram_tensor("bz", [128, D], FP, kind="ExternalInput").ap()
    # y[p, g*D + f] = out[token g*128 + p, f]
    y_d = nc.dram_tensor("y", [128, (L // 128) * D], F16,
                         kind="ExternalOutput").ap()

    AF = mybir.ActivationFunctionType
    ALU = mybir.AluOpType

    with tile.TileContext(nc) as tc, ExitStack() as ctx:
        const = ctx.enter_context(tc.tile_pool(name="const", bufs=1))
        tp = ctx.enter_context(tc.tile_pool(name="t", bufs=3))
        s1p = ctx.enter_context(tc.tile_pool(name="s1", bufs=2))
        s2p = ctx.enter_context(tc.tile_pool(name="s2", bufs=2))
        zp = ctx.enter_context(tc.tile_pool(name="z", bufs=2))
        stat = ctx.enter_context(tc.tile_pool(name="stat", bufs=1))
        nwt = ctx.enter_context(tc.tile_pool(name="nwt", bufs=2))
        op_ = ctx.enter_context(tc.tile_pool(name="o", bufs=2))
        ps = ctx.enter_context(tc.tile_pool(name="ps", bufs=1, space="PSUM"))

        # wzt on the idle gpsimd (SWDGE) queue so the x stream starts at t~0
        wzt = const.tile([128, KT * D], F16)
        nc.gpsimd.dma_start(wzt[:].rearrange("p (k f) -> p k f", k=KT),
                            wzt_d.rearrange("k p f -> p k f"))
        if use_gb:
            gb = const.tile([128, 2 * D], FP)
            nc.gpsimd.dma_start(gb[:], gb_d[:])
        if use_bz:
            bzt = const.tile([128, D], FP)
            nc.gpsimd.dma_start(bzt[:], bz_d[:])

        # Warm the gelu_and_others ACT table set (Gelu+Square+Identity).
        warm = const.tile([128, 2], FP)
        nc.vector.memset(warm[:, 0:1], 0.0)
        nc.scalar.activation(warm[:, 1:2], warm[:, 0:1], AF.Gelu)

        NG = L // 128
        st = stat.tile([128, 2 * NG], FP)   # S1 cols 0..7, S2 cols 8..15

        # Pre-warm the PE HAM clock gate with dummy matmuls on wzt while
        # the x stream is still in flight (PE is otherwise idle until the
        # first real MM at ~23us; cold PE runs MMs at half clock).
        pwarm = ps.tile([128, D], FP, tag="ps0", name="pwarm")
        for _ in range(16):
            nc.tensor.matmul(pwarm[:], wzt[:, 0:128], wzt[:, 0:D],
                             start=True, stop=True)

        # ---- loads: per phase, two k-halves on the sync/scalar queues ----
        t_tiles = []
        for h in range(PH):
            en = E * NPH[h]
            t = tp.tile([128, KT * en], F16, tag="t", name="t")
            tv = t[:].rearrange("p (k en) -> p k en", k=KT)
            nc.sync.dma_start(
                tv[:, 0:2, :],
                x_d[h][0:256, :].rearrange("(k p) en -> p k en", p=128))
            nc.sync.dma_start(
                tv[:, 2:4, :],
                x_d[h][256:512, :].rearrange("(k p) en -> p k en", p=128))
            t_tiles.append(t)

        g0 = 0
        for h in range(PH):
            np_, en = NPH[h], E * NPH[h]
            gp = np_ // 128
            t = t_tiles[h]
            tv = t[:].rearrange("p (k x) -> p k x", k=KT)
            # lvl1: 8 experts -> 4 partial sums (fp16 2x mode)
            s1 = s1p.tile([128, KT * 4 * np_], F16, tag="s1", name="s1")
            s1v = s1[:].rearrange("p (k x) -> p k x", k=KT)
            nc.vector.tensor_add(s1v[:, :, :],
                                 tv[:, :, 0:4 * np_], tv[:, :, 4 * np_:8 * np_])
            # lvl2: 4 -> 2
            s2 = s2p.tile([128, KT * 2 * np_], F16, tag="s2", name="s2")
            s2v = s2[:].rearrange("p (k x) -> p k x", k=KT)
            nc.vector.tensor_add(s2v[:, :, :],
                                 s1v[:, :, 0:2 * np_], s1v[:, :, 2 * np_:4 * np_])

            zs = []
            for gl in range(gp):
                g = g0 + gl
                psz = ps.tile([128, D], FP, tag=f"ps{g}", name=f"ps{g}")
                mi = 0
                for k in range(KT):
                    for q in range(2):
                        nc.tensor.matmul(
                            psz[:],
                            s2[:, k * 2 * np_ + q * np_ + gl * 128:
                               k * 2 * np_ + q * np_ + (gl + 1) * 128],
                            wzt[:, k * D:(k + 1) * D],
                            start=(mi == 0), stop=(mi == 2 * KT - 1),
                        )
                        mi += 1
                if use_bz:
                    nc.vector.tensor_add(psz[:], psz[:], bzt[:])

                z = zp.tile([128, D], F16, tag=f"z{gl}", name=f"z{gl}")
                nc.scalar.activation(z[:], psz[:], AF.Gelu,
                                     accum_out=st[:, g:g + 1])
                nc.scalar.activation(psz[:], z[:], AF.Square,
                                     accum_out=st[:, NG + g:NG + g + 1])
                zs.append(z)

            # ---- batched LN stats for this phase (on the idle GpSimd) ------
            nb = nwt.tile([128, 6 * gp], FP, tag="nb", name="nb")
            mn = nb[:, 0:gp]              # -mu
            ve = nb[:, gp:2 * gp]
            msq = nb[:, 2 * gp:3 * gp]
            y0 = nb[:, 3 * gp:4 * gp]     # -> rstd
            t1 = nb[:, 4 * gp:5 * gp]
            nmr = nb[:, 5 * gp:6 * gp]    # -mu*rstd
            V = nc.vector
            V.tensor_scalar(mn, st[:, g0:g0 + gp], -1.0 / D, None, ALU.mult)
            V.tensor_scalar(ve, st[:, NG + g0:NG + g0 + gp], 1.0 / D,
                            LN_EPS, ALU.mult, ALU.add)
            V.tensor_mul(msq, mn, mn)
            V.tensor_sub(ve, ve, msq)
            V.tensor_scalar(y0.bitcast(I32), ve.bitcast(I32),
                            1, None, ALU.logical_shift_right)
            V.tensor_scalar(y0.bitcast(I32), y0.bitcast(I32),
                            0x5F3759DF, -1, ALU.subtract, ALU.mult)
            V.tensor_mul(t1, y0, y0)
            V.tensor_mul(t1, t1, ve)
            V.tensor_scalar(t1, t1, -0.5, 1.5, ALU.mult, ALU.add)
            V.tensor_mul(y0, t1, y0)      # y0 <- rstd
            V.tensor_mul(nmr, mn, y0)     # nmr <- -mu*rstd

            # ---- normalize on ACT (Identity, same table set) + store -------
            o = op_.tile([128, gp * D], F16, tag="o", name="o")
            for gl in range(gp):
                nc.vector.tensor_scalar(o[:, gl * D:(gl + 1) * D], zs[gl][:],
                                        y0[:, gl:gl + 1], nmr[:, gl:gl + 1],
                                        ALU.mult, ALU.add)
                if use_gb:
                    nc.vector.tensor_mul(o[:, gl * D:(gl + 1) * D],
                                         o[:, gl * D:(gl + 1) * D], gb[:, 0:D])
                    nc.vector.tensor_add(o[:, gl * D:(gl + 1) * D],
                                         o[:, gl * D:(gl + 1) * D],
                                         gb[:, D:2 * D])
            nc.sync.dma_start(y_d[:, g0 * D:(g0 + gp) * D], o[:])
            g0 += gp

    nc.compile()
    return nc


def get_nc(use_gb: bool, use_bz: bool):
    key = (use_gb, use_bz)
    if key not in _CACHE:
        _CACHE[key] = _build(use_gb, use_bz)
    return _CACHE[key]


def _host_prep(hgnn_w, hgnn_b, comb_w, comb_b, ln_gamma, ln_beta):
    W0, W1 = hgnn_w[0].astype(np.float64), hgnn_w[1].astype(np.float64)
    b0, b1 = hgnn_b[0].astype(np.float64), hgnn_b[1].astype(np.float64)
    Wz = comb_w.astype(np.float64) @ W1 @ W0
    bz = (b0 @ W1.T + b1) @ comb_w.T.astype(np.float64) + comb_b
    wzt = np.ascontiguousarray((Wz / 8.0).T.astype(np.float16)
                               .reshape(KT, 128, D))
    bz = bz.astype(np.float32)

    use_bz = bool(np.any(bz != 0))
    use_gb = bool(np.any(ln_gamma != 1) or np.any(ln_beta != 0))
    gb = np.concatenate([
        np.broadcast_to(ln_gamma.astype(np.float32), (128, D)),
        np.broadcast_to(ln_beta.astype(np.float32), (128, D)),
    ], axis=1).copy()
    bzb = np.broadcast_to(bz, (128, D)).copy()
    return wzt, gb, bzb, use_gb, use_bz


def _stage_x(x_core):
    """[1024 n, 8 e, 512 d] f32 -> per-phase planes [512 d, 8 e * nph] f16
    with plane[h][d, e*nph + n'] = x[OFF[h] + n', e, d]."""
    x16 = np.asarray(x_core, np.float32).astype(np.float16)
    planes = []
    for h in range(PH):
        xs = x16[OFF[h]:OFF[h] + NPH[h]]          # [nph, E, D]
        planes.append(np.ascontiguousarray(xs.transpose(2, 1, 0))
                      .reshape(D, E * NPH[h]))
    return planes


def _unstage_y(y):
    """y [128 p, NG*D] f16 -> [1024 tok, 512] f32."""
    out = np.asarray(y, np.float16).reshape(128, L // 128, D)
    return np.ascontiguousarray(out.transpose(1, 0, 2)).reshape(L, D)


def kernel(expert_outputs, hgnn_w, hgnn_b, comb_w, comb_b, ln_gamma, ln_beta,
           nodes_idx, edges_idx):
    expert_outputs = np.asarray(expert_outputs, np.float32)
    wzt, gb, bzb, use_gb, use_bz = _host_prep(
        np.asarray(hgnn_w, np.float32), np.asarray(hgnn_b, np.float32),
        np.asarray(comb_w, np.float32), np.asarray(comb_b, np.float32),
        np.asarray(ln_gamma, np.float32), np.asarray(ln_beta, np.float32))

    nc = get_nc(use_gb, use_bz)

    in_maps = []
    for c in range(N_CORES):
        planes = _stage_x(expert_outputs[c])
        m = {"wzt": wzt}
        for h in range(PH):
            m[f"x{h}"] = planes[h]
        if use_gb:
            m["gb"] = gb
        if use_bz:
            m["bz"] = bzb
        in_maps.append(m)

    res = run_bass_kernel_spmd(nc, in_maps, list(range(N_CORES)))
    out = np.stack([_unstage_y(res.results[c]["y"]) for c in range(N_CORES)],
                   axis=0)
    return out.astype(np.float32)


# revision 24
# speedup vs baseline: 1.0292x; 1.0292x over previous
"""Trainium2 Bass kernel for nn_HGNNExpertCoupler (B=8, L=1024, E=8, D=512).

Math: the all-pairs hypergraph operator D^-1 H B^-1 H^T has unit column
sums, so it preserves the expert-mean, and the whole network collapses to

    out = LN(gelu(mean_E(x) @ Wz^T + bz)) * gamma + beta
    Wz  = Wc @ W1 @ W0,  bz = (b0 @ W1^T + b1) @ Wc^T + bc

Per-core layout (data parallel on B, one batch row per core, 1024 tokens):

  x is staged on host as fp16, d-major, one DRAM plane per token phase
  (uneven 384/256/256/128 split so the tail chain after the last byte is
  short).  Loads are plain full-rate HWDGE transfers on the sync queue
  (2-6 KB descriptors); wzt rides the otherwise-idle gpsimd queue.  A
  burst of dummy matmuls on wzt pre-warms the PE HAM clock gate while
  the stream is in flight.  The expert reduction is a 2-level fp16
  tensor_tensor tree on DVE (8 -> 4 -> 2 partial sums); the final 2-way
  fold is absorbed into the matmul accumulation (8 MMs per 128-token
  group).  ACT applies Gelu (accum_out = per-token sum z) and Square
  (accum_out = sum z^2, output recycled into the group's psum bank);
  both live in the gelu_and_others table set (one table load, warmed at
  t~0).  LayerNorm finishes on DVE: per-phase batched quake-rsqrt + 1
  Newton step, then one fp16 tensor_scalar normalize per group.
  Outputs are written fp16 and upcast on host.
"""

import os
import sys

import numpy as np

for _p in ("/opt/trn_rl_repo", "/opt/trn_rl_repo/pypackages",
           "/root/.axon_site/_ro/trn_rl_repo",
           "/root/.axon_site/_ro/pypackages"):
    if os.path.isdir(_p) and _p not in sys.path:
        sys.path.append(_p)

from contextlib import ExitStack

import concourse.bass as bass
import concourse.tile as tile
from concourse import bacc, mybir
from concourse.bass_utils import run_bass_kernel_spmd

FP = mybir.dt.float32
F16 = mybir.dt.float16
I32 = mybir.dt.int32

B, L, E, D = 8, 1024, 8, 512
KT = D // 128                     # 4 contraction k-blocks
NPH = [384, 256, 256, 128]        # tokens per phase
OFF = [0, 384, 640, 896]
PH = len(NPH)
LN_EPS = 1e-5
N_CORES = 8

_CACHE = {}


def _build(use_gb: bool, use_bz: bool):
    nc = bacc.Bacc("TRN2", target_bir_lowering=False, debug=False,
                   num_devices=N_CORES)

    x_d = [nc.dram_tensor(f"x{h}", [D, E * NPH[h]], F16,
                          kind="ExternalInput").ap()
           for h in range(PH)]
    wzt_d = nc.dram_tensor("wzt", [KT, 128, D], F16, kind="ExternalInput").ap()
    if use_gb:
        gb_d = nc.dram_tensor("gb", [128, 2 * D], FP, kind="ExternalInput").ap()
    if use_bz:
        bz_d = nc.dram_tensor("bz", [128, D], FP, kind="ExternalInput").ap()
    # y[p, g*D + f] = out[token g*128 + p, f]
    y_d = nc.dram_tensor("y", [128, (L // 128) * D], F16,
                         kind="ExternalOutput").ap()

    AF = mybir.ActivationFunctionType
    ALU = mybir.AluOpType

    with tile.TileContext(nc) as tc, ExitStack() as ctx:
        const = ctx.enter_context(tc.tile_pool(name="const", bufs=1))
        tp = ctx.enter_context(tc.tile_pool(name="t", bufs=3))
        s1p = ctx.enter_context(tc.tile_pool(name="s1", bufs=2))
        s2p = ctx.enter_context(tc.tile_pool(name="s2", bufs=2))
        zp = ctx.enter_context(tc.tile_pool(name="z", bufs=2))
        stat = ctx.enter_context(tc.tile_pool(name="stat", bufs=1))
        nwt = ctx.enter_context(tc.tile_pool(name="nwt", bufs=2))
        op_ = ctx.enter_context(tc.tile_pool(name="o", bufs=2))
        ps = ctx.enter_context(tc.tile_pool(name="ps", bufs=1, space="PSUM"))

        # wzt on the idle gpsimd (SWDGE) queue so the x stream starts at t~0
        wzt = const.tile([128, KT * D], F16)
        nc.gpsimd.dma_start(wzt[:].rearrange("p (k f) -> p k f", k=KT),
                            wzt_d.rearrange("k p f -> p k f"))
        if use_gb:
            gb = const.tile([128, 2 * D], FP)
            nc.gpsimd.dma_start(gb[:], gb_d[:])
        if use_bz:
            bzt = const.tile([128, D], FP)
            nc.gpsimd.dma_start(bzt[:], bz_d[:])

        # Warm the gelu_and_others ACT table set (Gelu+Square+Identity).
        warm = const.tile([128, 2], FP)
        nc.vector.memset(warm[:, 0:1], 0.0)
        nc.scalar.activation(warm[:, 1:2], warm[:, 0:1], AF.Gelu)

        NG = L // 128
        st = stat.tile([128, 2 * NG], FP)   # S1 cols 0..7, S2 cols 8..15

        # Pre-warm the PE HAM clock gate with dummy matmuls on wzt while
        # the x stream is still in flight (PE is otherwise idle until the
        # first real MM at ~23us; cold PE runs MMs at half clock).
        pwarm = ps.tile([128, D], FP, tag="ps0", name="pwarm")
        for _ in range(24):
            nc.tensor.matmul(pwarm[:], wzt[:, 0:128], wzt[:, 0:D],
                             start=True, stop=True)

        # ---- loads: per phase, two k-halves on the sync/scalar queues ----
        t_tiles = []
        for h in range(PH):
            en = E * NPH[h]
            t = tp.tile([128, KT * en], F16, tag="t", name="t")
            tv = t[:].rearrange("p (k en) -> p k en", k=KT)
            nc.sync.dma_start(
                tv[:, 0:2, :],
                x_d[h][0:256, :].rearrange("(k p) en -> p k en", p=128))
            nc.sync.dma_start(
                tv[:, 2:4, :],
                x_d[h][256:512, :].rearrange("(k p) en -> p k en", p=128))
            t_tiles.append(t)

        g0 = 0
        for h in range(PH):
            np_, en = NPH[h], E * NPH[h]
            gp = np_ // 128
            t = t_tiles[h]
            tv = t[:].rearrange("p (k x) -> p k x", k=KT)
            # lvl1: 8 experts -> 4 partial sums (fp16 2x mode)
            s1 = s1p.tile([128, KT * 4 * np_], F16, tag="s1", name="s1")
            s1v = s1[:].rearrange("p (k x) -> p k x", k=KT)
            nc.vector.tensor_add(s1v[:, :, :],
                                 tv[:, :, 0:4 * np_], tv[:, :, 4 * np_:8 * np_])
            # lvl2: 4 -> 2
            s2 = s2p.tile([128, KT * 2 * np_], F16, tag="s2", name="s2")
            s2v = s2[:].rearrange("p (k x) -> p k x", k=KT)
            nc.vector.tensor_add(s2v[:, :, :],
                                 s1v[:, :, 0:2 * np_], s1v[:, :, 2 * np_:4 * np_])

            zs = []
            for gl in range(gp):
                g = g0 + gl
                psz = ps.tile([128, D], FP, tag=f"ps{g}", name=f"ps{g}")
                mi = 0
                for k in range(KT):
                    for q in range(2):
                        nc.tensor.matmul(
                            psz[:],
                            s2[:, k * 2 * np_ + q * np_ + gl * 128:
                               k * 2 * np_ + q * np_ + (gl + 1) * 128],
                            wzt[:, k * D:(k + 1) * D],
                            start=(mi == 0), stop=(mi == 2 * KT - 1),
                        )
                        mi += 1
                if use_bz:
                    nc.vector.tensor_add(psz[:], psz[:], bzt[:])

                z = zp.tile([128, D], F16, tag=f"z{gl}", name=f"z{gl}")
                nc.scalar.activation(z[:], psz[:], AF.Gelu,
                                     accum_out=st[:, g:g + 1])
                nc.scalar.activation(psz[:], z[:], AF.Square,
                                     accum_out=st[:, NG + g:NG + g + 1])
                zs.append(z)

            # ---- batched LN stats for this phase (on the idle GpSimd) ------
            nb = nwt.tile([128, 6 * gp], FP, tag="nb", name="nb")
            mn = nb[:, 0:gp]              # -mu
            ve = nb[:, gp:2 * gp]
            msq = nb[:, 2 * gp:3 * gp]
            y0 = nb[:, 3 * gp:4 * gp]     # -> rstd
            t1 = nb[:, 4 * gp:5 * gp]
            nmr = nb[:, 5 * gp:6 * gp]    # -mu*rstd
            V = nc.vector
            V.tensor_scalar(mn, st[:, g0:g0 + gp], -1.0 / D, None, ALU.mult)
            V.tensor_scalar(ve, st[:, NG + g0:NG + g0 + gp], 1.0 / D,
                            LN_EPS, ALU.mult, ALU.add)
            V.tensor_mul(msq, mn, mn)
            V.tensor_sub(ve, ve, msq)
            V.tensor_scalar(y0.bitcast(I32), ve.bitcast(I32),
                            1, None, ALU.logical_shift_right)
            V.tensor_scalar(y0.bitcast(I32), y0.bitcast(I32),
                            0x5F3759DF, -1, ALU.subtract, ALU.mult)
            V.tensor_mul(t1, y0, y0)
            V.tensor_mul(t1, t1, ve)
            V.tensor_scalar(t1, t1, -0.5, 1.5, ALU.mult, ALU.add)
            V.tensor_mul(y0, t1, y0)      # y0 <- rstd
            V.tensor_mul(nmr, mn, y0)     # nmr <- -mu*rstd

            # ---- normalize on ACT (Identity, same table set) + store -------
            o = op_.tile([128, gp * D], F16, tag="o", name="o")
            for gl in range(gp):
                nc.vector.tensor_scalar(o[:, gl * D:(gl + 1) * D], zs[gl][:],
                                        y0[:, gl:gl + 1], nmr[:, gl:gl + 1],
                                        ALU.mult, ALU.add)
                if use_gb:
                    nc.vector.tensor_mul(o[:, gl * D:(gl + 1) * D],
                                         o[:, gl * D:(gl + 1) * D], gb[:, 0:D])
                    nc.vector.tensor_add(o[:, gl * D:(gl + 1) * D],
                                         o[:, gl * D:(gl + 1) * D],
                                         gb[:, D:2 * D])
            nc.sync.dma_start(y_d[:, g0 * D:(g0 + gp) * D], o[:])
            g0 += gp

    nc.compile()
    return nc


def get_nc(use_gb: bool, use_bz: bool):
    key = (use_gb, use_bz)
    if key not in _CACHE:
        _CACHE[key] = _build(use_gb, use_bz)
    return _CACHE[key]


def _host_prep(hgnn_w, hgnn_b, comb_w, comb_b, ln_gamma, ln_beta):
    W0, W1 = hgnn_w[0].astype(np.float64), hgnn_w[1].astype(np.float64)
    b0, b1 = hgnn_b[0].astype(np.float64), hgnn_b[1].astype(np.float64)
    Wz = comb_w.astype(np.float64) @ W1 @ W0
    bz = (b0 @ W1.T + b1) @ comb_w.T.astype(np.float64) + comb_b
    wzt = np.ascontiguousarray((Wz / 8.0).T.astype(np.float16)
                               .reshape(KT, 128, D))
    bz = bz.astype(np.float32)

    use_bz = bool(np.any(bz != 0))
    use_gb = bool(np.any(ln_gamma != 1) or np.any(ln_beta != 0))
    gb = np.concatenate([
        np.broadcast_to(ln_gamma.astype(np.float32), (128, D)),
        np.broadcast_to(ln_beta.astype(np.float32), (128, D)),
    ], axis=1).copy()
    bzb = np.broadcast_to(bz, (128, D)).copy()
    return wzt, gb, bzb, use_gb, use_bz


def _stage_x(x_core):
    """[1024 n, 8 e, 512 d] f32 -> per-phase planes [512 d, 8 e * nph] f16
    with plane[h][d, e*nph + n'] = x[OFF[h] + n', e, d]."""
    x16 = np.asarray(x_core, np.float32).astype(np.float16)
    planes = []
    for h in range(PH):
        xs = x16[OFF[h]:OFF[h] + NPH[h]]          # [nph, E, D]
        planes.append(np.ascontiguousarray(xs.transpose(2, 1, 0))
                      .reshape(D, E * NPH[h]))
    return planes


def _unstage_y(y):
    """y [128 p, NG*D] f16 -> [1024 tok, 512] f32."""
    out = np.asarray(y, np.float16).reshape(128, L // 128, D)
    return np.ascontiguousarray(out.transpose(1, 0, 2)).reshape(L, D)


def kernel(expert_outputs, hgnn_w, hgnn_b, comb_w, comb_b, ln_gamma, ln_beta,
           nodes_idx, edges_idx):
    expert_outputs = np.asarray(expert_outputs, np.float32)
    wzt, gb, bzb, use_gb, use_bz = _host_prep(
        np.asarray(hgnn_w, np.float32), np.asarray(hgnn_b, np.float32),
        np.asarray(comb_w, np.float32), np.asarray(comb_b, np.float32),
        np.asarray(ln_gamma, np.float32), np.asarray(ln_beta, np.float32))

    nc = get_nc(use_gb, use_bz)

    in_maps = []
    for c in range(N_CORES):
        planes = _stage_x(expert_outputs[c])
        m = {"wzt": wzt}
        for h in range(PH):
            m[f"x{h}"] = planes[h]
        if use_gb:
            m["gb"] = gb
        if use_bz:
            m["bz"] = bzb
        in_maps.append(m)

    res = run_bass_kernel_spmd(nc, in_maps, list(range(N_CORES)))
    out = np.stack([_unstage_y(res.results[c]["y"]) for c in range(N_CORES)],
                   axis=0)
    return out.astype(np.float32)
